# revision 2
# baseline (speedup 1.0000x reference)
"""Trainium2 Bass kernel for 16-head causal MHA — v4 (HW-calibrated).

Sharding (as baseline): core c -> batch c//2, head-group c%2 (8 heads = 4
pairs).  Host sums the two partial [D, S] outputs per batch.

HW-calibrated design (v3 -> v4):
  - DoubleRow fp8 only where it doubles contraction per instruction: QKV
    projections (4 instead of 8 matmuls per 512-col block) and AV (one
    matmul per two t-tiles).  Scores run in plain fp16 (DR gave no
    per-column speedup on HW and cost accuracy + an extra copy).
  - qt/kt are single fp16 [128, 2048] tiles; the j==0 block (s<512) and
    K tiles 0..3 come from an fp16 sidecar projection of fp16 x (accuracy:
    short softmax rows dominate the absmax error), all other blocks from
    fp8 projections.
  - causal mask: scores matmul closes the accumulation (start&stop), then a
    128-col triangular -30000 matmul accumulates with skip_group_check.
  - exp on ACT -> fp8 wt (fp16 for j0); softmax denominator via the
    ones-block of V'; normalize = DVE copy + reciprocal_approx_fast + muls.
  - j-descending schedule (j=3 first): ACT's exp load is ~(j+1)-weighted,
    so the deep backlog keeps ACT saturated while the PE trails with
    projection / AV / out-projection fillers popped one per attention tile.
  - output DMA fp16; host accumulates fp32.
"""

import sys

for _p in ("/opt/trn_rl_repo", "/root/.axon_site/_ro/trn_rl_repo"):
    if _p not in sys.path:
        sys.path.insert(0, _p)

import os

import numpy as np
import ml_dtypes

import concourse.bacc as bacc
import concourse.mybir as mybir
from concourse import bass_utils
from concourse.masks import make_identity, make_causal_mask
from concourse.tile import TileContext

FILL_EVERY = int(os.environ.get("K4_FILL_EVERY", "1"))

P = 128
S = 2048
D = 1024
H = 16
DK = 64
B = 4
NCORES = 8
HPC = 8
NPAIR = 4
SB = 512
NSB = S // SB     # 4 j-blocks
TT = S // P       # 16 t-tiles
DT = D // P       # 8 d-tiles
NDP = DT // 2     # 4 DoubleRow d-steps
MASKV = -30000.0

F32 = mybir.dt.float32
F16 = mybir.dt.float16
F8 = mybir.dt.float8e4
AF = mybir.ActivationFunctionType
MUL = mybir.AluOpType.mult
DR = mybir.MatmulPerfMode.DoubleRow

NPF8 = ml_dtypes.float8_e4m3


def build_nc(debug=False):
    nc = bacc.Bacc()
    x8_in = nc.dram_tensor("x8", [P, NDP * 2 * S], F8, kind="ExternalInput")
    wq_in = nc.dram_tensor("wq8", [P, NPAIR * 1024], F8, kind="ExternalInput")
    wk_in = nc.dram_tensor("wk8", [P, NPAIR * 1024], F8, kind="ExternalInput")
    wv_in = nc.dram_tensor("wv8", [P, NPAIR * 1024], F8, kind="ExternalInput")
    bq_d = nc.dram_tensor("bq_n", [P, NPAIR], F32, kind="ExternalInput")
    bk_d = nc.dram_tensor("bk_n", [P, NPAIR], F32, kind="ExternalInput")
    bv_d = nc.dram_tensor("bv_n", [P, NPAIR], F32, kind="ExternalInput")
    # fp16 sidecar: x cols 0:512 and per-pair Q/K/V weights
    xj0_in = nc.dram_tensor("xj0", [D, SB], F16, kind="ExternalInput")
    wq6_in = nc.dram_tensor("wq6", [D, NPAIR * P], F16, kind="ExternalInput")
    wk6_in = nc.dram_tensor("wk6", [D, NPAIR * P], F16, kind="ExternalInput")
    wv6_in = nc.dram_tensor("wv6", [D, NPAIR * P], F16, kind="ExternalInput")
    wo_t = nc.dram_tensor("wo_t", [HPC * DK, D], F16, kind="ExternalInput")
    out = nc.dram_tensor("out_part", [D, S], F16, kind="ExternalOutput")

    with TileContext(nc) as tc:
        from contextlib import ExitStack

        with ExitStack() as ctx:
            pool = lambda *a, **k: ctx.enter_context(tc.tile_pool(*a, **k))
            x_pool = pool(name="x", bufs=1)
            xj0_pool = pool(name="xj0", bufs=DT)
            wgt_pool = pool(name="wgt", bufs=3 * NPAIR)
            wgt6_pool = pool(name="wgt6", bufs=3 * NPAIR)
            wo_pool = pool(name="wo", bufs=NPAIR)
            qt_pool = pool(name="qt", bufs=NPAIR)
            kt_pool = pool(name="kt", bufs=NPAIR)
            vp_pool = pool(name="vp", bufs=NPAIR)
            vp6_pool = pool(name="vp6", bufs=NPAIR)
            vstg_pool = pool(name="vstg", bufs=3)
            wt_pool = pool(name="wt", bufs=5)
            wt6_pool = pool(name="wt6", bufs=2)
            ot_pool = pool(name="ot", bufs=NPAIR)
            rcs_pool = pool(name="rcs", bufs=3)
            ost_pool = pool(name="ost", bufs=3)
            const_pool = pool(name="const", bufs=1)
            ps_big = pool(name="ps_big", bufs=3, space="PSUM")  # sc + pa
            ps_ms = pool(name="ps_ms", bufs=2, space="PSUM")

            ident = const_pool.tile([P, P], F16)
            make_identity(nc, ident[:])
            cmask = const_pool.tile([P, P], F16)
            make_causal_mask(nc, cmask[:], mask_val=MASKV)
            biases = {}
            for nm, src in (("q", bq_d), ("k", bk_d), ("v", bv_d)):
                t = const_pool.tile([P, NPAIR], F32, name=f"b{nm}")
                nc.sync.dma_start(t[:], src[:])
                biases[nm] = t

            # DMA order = first-use order: wk6+xj0 (kt block 0), x8+wq8
            # (q3), then the rest; wq6/wv6 (j0 phase, runs last) at the end
            w6 = {}

            def load_w6(nm, srcw):
                for p in range(NPAIR):
                    t = wgt6_pool.tile([P, DT * P], F16, tag="wgt6",
                                       name=f"w6{nm}{p}")
                    nc.sync.dma_start(
                        t[:].rearrange("r (d c) -> r d c", d=DT),
                        srcw[:, p * P : (p + 1) * P].rearrange(
                            "(d r) c -> r d c", r=P),
                    )
                    w6[(nm, p)] = t

            load_w6("k", wk6_in)
            xj0 = []
            for d in range(DT):
                t = xj0_pool.tile([P, SB], F16, tag="xj0", name=f"xj0_{d}")
                nc.sync.dma_start(t[:], xj0_in[d * P : (d + 1) * P, :])
                xj0.append(t)
            xt = x_pool.tile([P, NDP, 2, S], F8, name="x8")
            for dp in range(NDP):
                nc.sync.dma_start(
                    xt[:, dp, :, :].rearrange("p b c -> p (b c)"),
                    x8_in[:, dp * 2 * S : (dp + 1) * 2 * S],
                )
            w8 = {}
            for nm, srcw in (("q", wq_in), ("k", wk_in), ("v", wv_in)):
                for p in range(NPAIR):
                    t = wgt_pool.tile([P, NDP, 2, P], F8, tag="wgt",
                                      name=f"w8{nm}{p}")
                    nc.sync.dma_start(
                        t[:].rearrange("p a b c -> p (a b c)"),
                        srcw[:, p * 1024 : (p + 1) * 1024],
                    )
                    w8[(nm, p)] = t
            load_w6("q", wq6_in)
            load_w6("v", wv6_in)
            wo_tiles = []
            for p in range(NPAIR):
                t = wo_pool.tile([P, D], F16, tag="wo", name=f"wo{p}")
                nc.sync.dma_start(t[:], wo_t[p * P : (p + 1) * P, :])
                wo_tiles.append(t)

            # persistent per-pair tensors
            qts = [qt_pool.tile([P, S], F16, tag="qt", name=f"qt{p}")
                   for p in range(NPAIR)]
            kts = [kt_pool.tile([P, S], F16, tag="kt", name=f"kt{p}")
                   for p in range(NPAIR)]
            # vp8[p]: [128 t, 2 heads, 8 tile-pairs, 2 slots, 128 (V|ones)]
            vp8 = [vp_pool.tile([P, 2, TT // 2, 2, P], F8, tag="vp8",
                                name=f"vp8_{p}") for p in range(NPAIR)]
            vp6 = [vp6_pool.tile([P, 2, 4, P], F16, tag="vp6",
                                 name=f"vp6_{p}") for p in range(NPAIR)]
            for p in range(NPAIR):
                nc.gpsimd.memset(vp8[p][:], 1.0)
                nc.gpsimd.memset(vp6[p][:], 1.0)
            ots = [ot_pool.tile([P, S], F16, tag="ot", name=f"ot{p}")
                   for p in range(NPAIR)]

            # ---------- units ----------
            def vfill(p, j, vst, f16):
                """transpose vst [128,512] into vp slots for j's 4 t-tiles."""
                for u2 in range(2):
                    pt2 = ps_ms.tile([P, 2 * P], F16, tag="ms", name="pt2")
                    for q in range(2):
                        u = 2 * u2 + q
                        nc.tensor.transpose(
                            pt2[:, q * P : (q + 1) * P],
                            vst[:, u * P : (u + 1) * P],
                            ident[:],
                        )
                    tg = 4 * j + 2 * u2
                    # one copy per pt2: dst [h, slot, 64], src [h, q, 64]
                    if f16:
                        dst = vp6[p][:, :, 2 * u2 : 2 * u2 + 2, 0:DK]
                    else:
                        dst = vp8[p][:, :, tg // 2, :, 0:DK]
                    nc.vector.tensor_copy(
                        dst,
                        pt2[:].rearrange("p (t h c) -> p h t c", t=2, h=2),
                    )

            def proj8_unit(nm, p, j):
                ps = ps_ms.tile([P, SB], F32, tag="ms", name=f"ps8{nm}")
                for dp in range(NDP):
                    nc.tensor.matmul(
                        ps[:],
                        w8[(nm, p)][:, dp, :, :],
                        xt[:, dp, :, j * SB : (j + 1) * SB],
                        start=(dp == 0),
                        stop=(dp == NDP - 1),
                        perf_mode=DR,
                    )
                if nm == "v":
                    vst = vstg_pool.tile([P, SB], F16, tag="vstg")
                    nc.vector.tensor_scalar_add(
                        vst[:], ps[:], biases["v"][:, p : p + 1])
                    vfill(p, j, vst, f16=False)
                else:
                    dest = qts[p] if nm == "q" else kts[p]
                    nc.vector.tensor_scalar_add(
                        dest[:, j * SB : (j + 1) * SB],
                        ps[:],
                        biases[nm][:, p : p + 1],
                    )

            def proj6_unit(nm, p):
                """fp16 sidecar projection of s-block 0."""
                ps = ps_ms.tile([P, SB], F32, tag="ms", name=f"ps6{nm}")
                for d in range(DT):
                    nc.tensor.matmul(
                        ps[:],
                        w6[(nm, p)][:, d * P : (d + 1) * P],
                        xj0[d][:],
                        start=(d == 0),
                        stop=(d == DT - 1),
                    )
                if nm == "v":
                    vst = vstg_pool.tile([P, SB], F16, tag="vstg")
                    nc.vector.tensor_scalar_add(
                        vst[:], ps[:], biases["v"][:, p : p + 1])
                    vfill(p, 0, vst, f16=True)
                else:
                    dest = qts[p] if nm == "q" else kts[p]
                    nc.vector.tensor_scalar_add(
                        dest[:, 0:SB], ps[:], biases[nm][:, p : p + 1])

            def outproj_unit(j, m):
                ps = ps_ms.tile([P, SB], F32, tag="ms", name="ps_o")
                for p in range(NPAIR):
                    nc.tensor.matmul(
                        ps[:],
                        wo_tiles[p][:, m * P : (m + 1) * P],
                        ots[p][:, j * SB : (j + 1) * SB],
                        start=(p == 0),
                        stop=(p == NPAIR - 1),
                    )
                st = ost_pool.tile([P, SB], F16, tag="ost")
                nc.vector.tensor_copy(st[:], ps[:])
                nc.sync.dma_start(
                    out[m * P : (m + 1) * P, j * SB : (j + 1) * SB], st[:]
                )

            def outproj_units(j):
                return [(lambda m=m: outproj_unit(j, m)) for m in range(DT)]

            def attention_j(p, j, fillers, fill_ctr):
                f16 = j == 0
                nt = 4 * j + 4
                pa = ps_big.tile([P, 2, SB], F32, tag="big", name=f"pa{p}")
                wts = {}
                pend = []

                def av_emit(i):
                    diag = i >= 4 * j
                    r = i - 4 * j
                    w = SB - P * r if diag else SB
                    q = i % 2
                    wt_t = wts[i // 2]
                    last = i == nt - 1
                    if diag:
                        c0 = P * r
                        for h in range(2):
                            vsl = (vp6[p][:, h, i, :] if f16
                                   else vp8[p][:, h, i // 2, q, :])
                            nc.tensor.matmul(
                                pa[:, h, c0:SB],
                                vsl,
                                wt_t[:, h, q, 0:w],
                                start=(i == 0),
                                stop=last,
                            )
                    elif q == 1:
                        for h in range(2):
                            nc.tensor.matmul(
                                pa[:, h, :],
                                vp8[p][:, h, i // 2, :, :],
                                wt_t[:, h, :, :],
                                start=(i == 1),
                                stop=False,
                                perf_mode=DR,
                            )

                for i in range(nt):
                    fill_ctr[0] += 1
                    if fillers and fill_ctr[0] % FILL_EVERY == 0:
                        fillers.pop(0)[1]()
                    diag = i >= 4 * j
                    r = i - 4 * j
                    w = SB - P * r if diag else SB
                    qoff = j * SB + (P * r if diag else 0)
                    q = i % 2
                    sc = ps_big.tile([P, 2, SB], F32, tag="big", name="sc")
                    for h in range(2):
                        nc.tensor.matmul(
                            sc[:, h, 0:w],
                            kts[p][64 * h : 64 * h + 64, i * P : (i + 1) * P],
                            qts[p][64 * h : 64 * h + 64, qoff : qoff + w],
                            start=True, stop=True,
                        )
                        if diag:
                            nc.tensor.matmul(
                                sc[:, h, 0:P],
                                cmask[:],
                                ident[:],
                                start=False, stop=True,
                                skip_group_check=True,
                            )
                    if q == 0:
                        if f16:
                            wt_cur = wt6_pool.tile([P, 2, 2, SB], F16,
                                                   tag="wt6", name="wt6c")
                        else:
                            wt_cur = wt_pool.tile([P, 2, 2, SB], F8,
                                                  tag="wt", name="wt8c")
                        wts[i // 2] = wt_cur
                    nc.scalar.activation(
                        wts[i // 2][:, :, q, 0:w], sc[:, :, 0:w], AF.Exp,
                        scale=0.125,
                    )
                    pend.append(i)
                    # defer AV by 2 tiles so the PE never waits on exp
                    if len(pend) > 2:
                        av_emit(pend.pop(0))
                for i in pend:
                    av_emit(i)
                den = rcs_pool.tile([64, 2, SB], F32, tag="den", name="den")
                nc.vector.tensor_copy(den[:], pa[DK:P, :, :])
                rcs = rcs_pool.tile([64, 2, SB], F32, tag="rcs", name="rcs")
                nc.vector.reciprocal_approx_fast(rcs[:], den[:])
                for h in range(2):
                    nc.vector.tensor_tensor(
                        ots[p][h * DK : (h + 1) * DK, j * SB : (j + 1) * SB],
                        pa[0:DK, h, :],
                        rcs[:, h, :],
                        MUL,
                    )

            # ---------- schedule ----------
            fillers = []
            fill_ctr = [0]

            def drain(pred):
                rest = []
                for tag, fn in fillers:
                    (fn() if pred(tag) else rest.append((tag, fn)))
                fillers[:] = rest

            for p in range(NPAIR):
                seq = [("j0k",), ("q", 3), ("kv", 0, "v"),
                       ("kv", 1, "k"), ("kv", 1, "v"),
                       ("kv", 2, "k"), ("kv", 2, "v"),
                       ("kv", 3, "k"), ("kv", 3, "v"),
                       ("q", 2), ("q", 1), ("j0q",), ("j0v",)]
                for tag in seq:
                    full = (tag[0], tag[1] if len(tag) > 1 else None,
                            tag[2] if len(tag) > 2 else None, p)
                    if tag[0] == "j0k":
                        fn = lambda p=p: proj6_unit("k", p)
                    elif tag[0] == "j0q":
                        fn = lambda p=p: proj6_unit("q", p)
                    elif tag[0] == "j0v":
                        fn = lambda p=p: proj6_unit("v", p)
                    elif tag[0] == "q":
                        fn = lambda p=p, jq=tag[1]: proj8_unit("q", p, jq)
                    else:
                        fn = (lambda p=p, b=tag[1], nm=tag[2]:
                              proj8_unit(nm, p, b))
                    fillers.append((full, fn))

            first = True
            for j in range(NSB - 1, -1, -1):
                for p in range(NPAIR):
                    if j == 0:
                        drain(lambda t, p=p: t[3] == p
                              and t[0] in ("j0q", "j0v"))
                    elif first:
                        drain(lambda t, p=p: t[3] == p
                              and t[0] in ("j0k",) or
                              (t[3] == p and t[0] == "q" and t[1] == 3))
                        first = False
                    else:
                        drain(lambda t, p=p, j=j: t[3] == p and (
                            t[0] == "j0k"
                            or (t[0] == "kv" and t[1] <= j)
                            or (t[0] == "q" and t[1] == j)))
                    attention_j(p, j, fillers, fill_ctr)
                fillers.extend((("op", j, None, m), u)
                               for m, u in enumerate(outproj_units(j)))
            drain(lambda t: True)

    nc.compile()
    return nc


_NC_CACHE = None


def _get_nc():
    global _NC_CACHE
    if _NC_CACHE is None:
        _NC_CACHE = build_nc()
    return _NC_CACHE


def _core_inputs(x, Wq, bq, Wk, bk, Wv, bv, Wo, c):
    b, g = c // 2, c % 2
    heads = list(range(g * HPC, (g + 1) * HPC))
    out = {}
    xT = np.ascontiguousarray(x[b].T)  # [D, S] f32
    x8 = xT.reshape(NDP, 2, P, S).transpose(2, 0, 1, 3)
    out["x8"] = np.ascontiguousarray(x8.reshape(P, NDP * 2 * S).astype(NPF8))

    def pack_w8(W):
        cols = np.empty((P, NPAIR, NDP, 2, P), dtype=np.float32)
        for p in range(NPAIR):
            hA, hB = heads[2 * p], heads[2 * p + 1]
            Wp = np.concatenate([W[hA], W[hB]], axis=1)  # [1024, 128]
            Wp = Wp.reshape(NDP, 2, P, P)
            cols[:, p] = Wp.transpose(2, 0, 1, 3)
        return np.ascontiguousarray(cols.reshape(P, NPAIR * 1024).astype(NPF8))

    def pack_b(bias):
        cols = np.empty((P, NPAIR), dtype=np.float32)
        for p in range(NPAIR):
            hA, hB = heads[2 * p], heads[2 * p + 1]
            cols[:, p] = np.concatenate([bias[hA], bias[hB]])
        return np.ascontiguousarray(cols)

    out["wq8"] = pack_w8(Wq)
    out["wk8"] = pack_w8(Wk)
    out["wv8"] = pack_w8(Wv)
    out["bq_n"] = pack_b(bq)
    out["bk_n"] = pack_b(bk)
    out["bv_n"] = pack_b(bv)
    out["xj0"] = np.ascontiguousarray(xT[:, 0:SB].astype(np.float16))

    def cat16(W):
        return np.ascontiguousarray(np.concatenate(
            [np.concatenate([W[heads[2 * p]], W[heads[2 * p + 1]]], axis=1)
             for p in range(NPAIR)], axis=1).astype(np.float16))
    out["wq6"] = cat16(Wq)
    out["wk6"] = cat16(Wk)
    out["wv6"] = cat16(Wv)
    out["wo_t"] = np.ascontiguousarray(
        Wo[:, g * HPC * DK : (g + 1) * HPC * DK].T.astype(np.float16)
    )
    return out


def kernel(x, Wq, bq, Wk, bk, Wv, bv, Wo, bo, _trace=False, _tmpdir=None):
    x = np.asarray(x, dtype=np.float32)
    nc = _get_nc()
    in_maps = [
        _core_inputs(x, Wq, bq, Wk, bk, Wv, bv, Wo, c) for c in range(NCORES)
    ]
    kw = {}
    if _trace:
        kw = dict(trace=True, tmpdir=_tmpdir)
    res = bass_utils.run_bass_kernel_spmd(
        nc, in_maps, core_ids=list(range(NCORES)), **kw
    )
    bo = np.asarray(bo, dtype=np.float32)
    out = np.empty((B, S, D), dtype=np.float32)
    for b in range(B):
        part = (res.results[2 * b]["out_part"].astype(np.float32)
                + res.results[2 * b + 1]["out_part"].astype(np.float32))
        out[b] = part.T + bo
    if _trace:
        kernel._last_results = res
    return out
